# revision 35
# baseline (speedup 1.0000x reference)
"""Trainium2 Bass kernel for nn_Attention_16707422781936.

Data-parallel over batch: B=8 -> one batch element per NeuronCore (8 cores).
Per core: qkv 1x1-conv GEMM, 8-head softmax attention over N=1600 tokens,
proj GEMM, depthwise 3x3 positional-encoding conv, summed output.
"""
import sys

sys.path.insert(0, "/opt/trn_rl_repo")

import ml_dtypes
import numpy as np

import concourse.bass as bass
import concourse.mybir as mybir
import concourse.tile as tile
from concourse import bacc
from concourse.bass_utils import run_bass_kernel_spmd

F32 = mybir.dt.float32
F32R = mybir.dt.float32r
BF16 = mybir.dt.bfloat16
ALU = mybir.AluOpType
EXP = mybir.ActivationFunctionType.Exp

C = 512          # channels
N = 1600         # tokens (40*40)
H = W = 40
NH = 8           # heads
KD = 32          # key dim
HD = 64          # head dim (v)
SCALE = KD ** -0.5

# n blocks (psum-bank sized) and m tiles (partition sized)
NB = [(0, 512), (512, 512), (1024, 512), (1536, 64)]
MT = [(i * 128, min(128, N - i * 128)) for i in range(13)]

_CACHE = {}


def build():
    nc = bacc.Bacc("TRN2", target_bir_lowering=False, debug=False,
                   enable_asserts=False)

    x_d = nc.dram_tensor("x", [C, N], BF16, kind="ExternalInput").ap()
    wqkvt_d = nc.dram_tensor("wqkvt", [128, 4 * 1024], BF16, kind="ExternalInput").ap()
    wprojt_d = nc.dram_tensor("wprojt", [128, 4 * 512], BF16, kind="ExternalInput").ap()
    bqk_d = nc.dram_tensor("bqk", [128, 4], F32, kind="ExternalInput").ap()
    bv_d = nc.dram_tensor("bv", [128, 4], F32, kind="ExternalInput").ap()
    bproj_d = nc.dram_tensor("bproj", [128, 4], F32, kind="ExternalInput").ap()
    bpe_d = nc.dram_tensor("bpe", [128, 4], F32, kind="ExternalInput").ap()
    wpe_d = nc.dram_tensor("wpe", [128, 36], F32, kind="ExternalInput").ap()
    ones_d = nc.dram_tensor("ones8", [128, NH], BF16, kind="ExternalInput").ap()
    out_d = nc.dram_tensor("out", [C, N], F32, kind="ExternalOutput").ap()

    with tile.TileContext(nc) as tc:
        with tc.tile_pool(name="persist", bufs=1) as per:
            qk_sb = per.tile([128, 4, N], BF16, tag="qk")      # q(h0-3),q(h4-7),k(h0-3),k(h4-7)
            v_sb = per.tile([128, 4, N], BF16, tag="v")         # v, channel-major (for dwconv)
            vt_sb = per.tile([128, 13, NH, HD + 1], BF16, tag="vt")  # v^T + ones col
            attn_sb = per.tile([128, 4, N], BF16, tag="attn")  # attention out, channel-major
            wprojt_sb = per.tile([128, 4, 512], BF16, tag="wprojt")
            bqk_sb = per.tile([128, 4], F32, tag="bqk")
            bv_sb = per.tile([128, 4], F32, tag="bv")
            bproj_sb = per.tile([128, 4], F32, tag="bproj")
            bpe_sb = per.tile([128, 4], F32, tag="bpe")
            wpe_sb = per.tile([128, 36], F32, tag="wpe")

            nc.sync.dma_start(wprojt_sb[:], wprojt_d.rearrange("p (t o) -> p t o", t=4))
            nc.sync.dma_start(bqk_sb[:], bqk_d)
            nc.sync.dma_start(bv_sb[:], bv_d)
            nc.sync.dma_start(bproj_sb[:], bproj_d)
            nc.sync.dma_start(bpe_sb[:], bpe_d)
            nc.sync.dma_start(wpe_sb[:], wpe_d)
            for mi in range(13):
                nc.sync.dma_start(vt_sb[:, mi, :, HD:HD + 1],
                                  ones_d.rearrange("p (h o) -> p h o", o=1))

            # ---------------- qkv GEMM phase ----------------
            with tc.tile_pool(name="ph_in", bufs=1) as ph_in:
                x_sb = ph_in.tile([128, 4, N], BF16, tag="x")
                wqkvt_sb = ph_in.tile([128, 4, 1024], BF16, tag="wqkvt")
                x_dr = x_d.rearrange("(t p) n -> p t n", p=128)
                w_dr = wqkvt_d.rearrange("p (t o) -> p t o", t=4)
                for kt in range(4):
                    nc.sync.dma_start(wqkvt_sb[:, kt], w_dr[:, kt])
                    for (n0, nw) in NB:
                        nc.sync.dma_start(x_sb[:, kt, n0:n0 + nw],
                                          x_dr[:, kt, n0:n0 + nw])

                with tc.tile_pool(name="ps_qkv", bufs=4, space="PSUM") as psq:
                    # q and k rows (qkv rows 0:512), output [o, n] layout
                    for mt in range(4):
                        for (n0, nw) in NB:
                            qp = psq.tile([128, 512], F32, tag="qp")
                            for kt in range(4):
                                nc.tensor.matmul(
                                    qp[0:128, 0:nw],
                                    wqkvt_sb[:, kt, mt * 128:(mt + 1) * 128],
                                    x_sb[:, kt, n0:n0 + nw],
                                    start=(kt == 0), stop=(kt == 3))
                            nc.vector.tensor_scalar(
                                out=qk_sb[:, mt, n0:n0 + nw], in0=qp[0:128, 0:nw],
                                scalar1=bqk_sb[:, mt:mt + 1], scalar2=None, op0=ALU.add)
                    # v rows (qkv rows 512:1024), channel-major, +bias (for dwconv)
                    for ct in range(4):
                        for (n0, nw) in NB:
                            qp = psq.tile([128, 512], F32, tag="qp")
                            for kt in range(4):
                                nc.tensor.matmul(
                                    qp[0:128, 0:nw],
                                    wqkvt_sb[:, kt, 512 + ct * 128:512 + (ct + 1) * 128],
                                    x_sb[:, kt, n0:n0 + nw],
                                    start=(kt == 0), stop=(kt == 3))
                            nc.vector.tensor_scalar(
                                out=v_sb[:, ct, n0:n0 + nw], in0=qp[0:128, 0:nw],
                                scalar1=bv_sb[:, ct:ct + 1], scalar2=None, op0=ALU.add)
                    # v^T (no bias; bias folded in post-AV add), token-major
                    for mi, (m0, mw) in enumerate(MT):
                        vp = psq.tile([128, 512], F32, tag="vp")
                        for kt in range(4):
                            nc.tensor.matmul(
                                vp[0:mw, 0:512],
                                x_sb[:, kt, m0:m0 + mw],
                                wqkvt_sb[:, kt, 512:1024],
                                start=(kt == 0), stop=(kt == 3))
                        nc.vector.tensor_copy(
                            out=vt_sb[0:mw, mi, :, 0:HD],
                            in_=vp[0:mw, 0:512].rearrange("p (h d) -> p h d", h=NH))

            # ---------------- depthwise 3x3 conv (VectorE) ----------------
            with tc.tile_pool(name="ph2", bufs=1) as ph2:
                pe_sb = ph2.tile([128, 4, H, W], F32, tag="pe")
                v4 = v_sb[:].rearrange("p t (h w) -> p t h w", h=H)

                # dwconv ops, generated lazily and dripped into attention
                def dwconv_ops():
                    for ct in range(4):
                        def center(ct=ct):
                            nc.vector.tensor_scalar(
                                out=pe_sb[:, ct], in0=v4[:, ct],
                                scalar1=wpe_sb[:, ct * 9 + 4:ct * 9 + 5],
                                scalar2=bpe_sb[:, ct:ct + 1],
                                op0=ALU.mult, op1=ALU.add)
                        yield center
                        for t in range(9):
                            dy, dx = t // 3 - 1, t % 3 - 1
                            if dy == 0 and dx == 0:
                                continue

                            def tap(ct=ct, t=t, dy=dy, dx=dx):
                                ys, ye = max(0, -dy), H - max(0, dy)
                                xs, xe = max(0, -dx), W - max(0, dx)
                                acc = pe_sb[:, ct, ys:ye, xs:xe]
                                nc.vector.scalar_tensor_tensor(
                                    out=acc,
                                    in0=v4[:, ct, ys + dy:ye + dy, xs + dx:xe + dx],
                                    scalar=wpe_sb[:, ct * 9 + t:ct * 9 + t + 1],
                                    in1=acc, op0=ALU.mult, op1=ALU.add)
                            yield tap

                # ---------------- attention + proj (flat pipeline) ----------
                pe3 = pe_sb[:].rearrange("p t h w -> p t (h w)")
                out_dr = out_d.rearrange("(t p) n -> p t n", p=128)
                with tc.tile_pool(name="ps_s", bufs=2, space="PSUM") as pss, \
                     tc.tile_pool(name="ps_av", bufs=3, space="PSUM") as psav, \
                     tc.tile_pool(name="ps_pj", bufs=1, space="PSUM") as pspj, \
                     tc.tile_pool(name="expp", bufs=3) as expp, \
                     tc.tile_pool(name="nrm", bufs=4) as nrm, \
                     tc.tile_pool(name="outp", bufs=3) as outp:

                    def proj_ops(nbi):
                        n0, nw = NB[nbi]
                        for ot in range(4):
                            pp = [None]
                            for kt in range(4):
                                def mm(ot=ot, kt=kt, pp=pp):
                                    if kt == 0:
                                        pp[0] = pspj.tile([128, 512], F32, tag="pp", name="pp")
                                    nc.tensor.matmul(
                                        pp[0][0:128, 0:nw],
                                        wprojt_sb[:, kt, ot * 128:(ot + 1) * 128],
                                        attn_sb[:, kt, n0:n0 + nw],
                                        start=(kt == 0), stop=(kt == 3))
                                yield mm

                            def evac(ot=ot, pp=pp):
                                ob = outp.tile([128, 512], F32, tag="ob")
                                nc.vector.scalar_tensor_tensor(
                                    out=ob[0:128, 0:nw], in0=pp[0][0:128, 0:nw],
                                    scalar=bproj_sb[:, ot:ot + 1],
                                    in1=pe3[:, ot, n0:n0 + nw],
                                    op0=ALU.add, op1=ALU.add)
                                nc.sync.dma_start(out_dr[:, ot, n0:n0 + nw],
                                                  ob[0:128, 0:nw])
                            yield evac

                    def normalize(p, n0, nw, avs):
                        for j in range(2):
                            drow = nrm.tile([1, 512], F32, tag="drow")
                            dsplit = nrm.tile([32, 16], F32, tag="dsplit")
                            rsplit = nrm.tile([32, 16], F32, tag="rsplit")
                            rc = nrm.tile([1, 512], F32, tag="rc")
                            rb = nrm.tile([HD, 512], F32, tag="rb")
                            nws = nw // 32
                            nc.vector.tensor_copy(drow[0:1, 0:nw],
                                                  avs[j][HD:HD + 1, 0:nw])
                            nc.sync.dma_start(dsplit[0:32, 0:nws], drow[0:1, 0:nw])
                            nc.vector.reciprocal(rsplit[0:32, 0:nws], dsplit[0:32, 0:nws])
                            nc.sync.dma_start(rc[0:1, 0:nw], rsplit[0:32, 0:nws])
                            nc.gpsimd.partition_broadcast(rb[0:HD, 0:nw], rc[0:1, 0:nw])
                            nc.vector.scalar_tensor_tensor(
                                out=attn_sb[j * 64:j * 64 + 64, p, n0:n0 + nw],
                                in0=avs[j][0:HD, 0:nw], scalar=1.0, in1=rb[0:HD, 0:nw],
                                op0=ALU.bypass, op1=ALU.mult)

                    import collections as _c
                    drip = _c.deque(dwconv_ops())   # PE-free DVE drips
                    pe_drip = _c.deque()            # PE drips (proj matmuls)
                    SWEEPS = [(nbi, p) for nbi in range(4) for p in range(4)]
                    pend = None  # deferred AV step: dict of sweep-step state

                    def emit_av(st):
                        for j in range(2):
                            nc.tensor.matmul(
                                st["avs"][j][0:HD + 1, 0:st["nw"]],
                                vt_sb[0:st["mw"], st["mi"], 2 * st["p"] + j, :],
                                st["es"][0:st["mw"], j * 512:j * 512 + st["nw"]],
                                start=(st["mi"] == 0), stop=(st["mi"] == 12))

                    proj_delay = _c.deque()

                    def retire(st):
                        emit_av(st)
                        if st["mi"] == 12:          # sweep finished
                            normalize(st["p"], st["n0"], st["nw"], st["avs"])
                            # release the previous nb's proj drip now that its
                            # normalize chains have had a sweep to complete
                            while proj_delay:
                                pe_drip.append(proj_delay.popleft())
                            if st["p"] == 3:        # all pairs done at this nb
                                proj_delay.extend(proj_ops(st["nbi"]))

                    for (nbi, p) in SWEEPS:
                        n0, nw = NB[nbi]
                        tq = p // 2
                        pb = (p % 2) * 64
                        avs = (psav.tile([HD + 1, 512], F32, tag="av", name="av0"),
                               psav.tile([HD + 1, 512], F32, tag="av", name="av1"))
                        for mi, (m0, mw) in enumerate(MT):
                            sp = pss.tile([128, 1024], F32, tag="sp")
                            for j in range(2):
                                nc.tensor.matmul(
                                    sp[0:mw, j * 512:j * 512 + nw],
                                    qk_sb[pb + 32 * j:pb + 32 * j + 32, 2 + tq, m0:m0 + mw],
                                    qk_sb[pb + 32 * j:pb + 32 * j + 32, tq, n0:n0 + nw],
                                    start=True, stop=True,
                                    tile_position=(pb + 32 * j, 0))
                            es = expp.tile([128, 1024], BF16, tag="es")
                            if nw == 512:
                                nc.scalar.activation(es[0:mw, :], sp[0:mw, :], EXP, scale=SCALE)
                            else:
                                sp3 = sp[:].rearrange("p (j n) -> p j n", j=2)
                                es3 = es[:].rearrange("p (j n) -> p j n", j=2)
                                nc.scalar.activation(es3[0:mw, :, 0:nw], sp3[0:mw, :, 0:nw],
                                                     EXP, scale=SCALE)
                            if pend is not None:
                                retire(pend)
                            if pe_drip:
                                pe_drip.popleft()()
                            elif drip:
                                drip.popleft()()
                            pend = dict(avs=avs, p=p, nw=nw, mi=mi, mw=mw,
                                        es=es, n0=n0, nbi=nbi)
                    retire(pend)
                    while proj_delay:
                        pe_drip.append(proj_delay.popleft())
                    while pe_drip:
                        pe_drip.popleft()()
                    while drip:
                        drip.popleft()()

    nc.compile()
    return nc


def _prep(Wqkv, bqkv, Wproj, bproj, Wpe, bpe):
    WqkvT = np.ascontiguousarray(Wqkv.T)            # [512, 1024]
    wqkvt_h = np.ascontiguousarray(
        WqkvT.reshape(4, 128, 1024).transpose(1, 0, 2).reshape(128, 4096)
    ).astype(ml_dtypes.bfloat16)
    WprojT = np.ascontiguousarray(Wproj.T)          # [512, 512]
    wprojt_h = np.ascontiguousarray(
        WprojT.reshape(4, 128, 512).transpose(1, 0, 2).reshape(128, 2048)
    ).astype(ml_dtypes.bfloat16)
    bqk_h = np.ascontiguousarray(bqkv[0:512].reshape(4, 128).T)
    bv_h = np.ascontiguousarray(bqkv[512:1024].reshape(4, 128).T)
    # attention out is produced WITHOUT the v bias; Wproj @ bv is a constant
    # per output channel, so fold it into the proj bias on the host
    bproj_eff = bproj + Wproj @ bqkv[512:1024]
    bproj_h = np.ascontiguousarray(bproj_eff.reshape(4, 128).T)
    bpe_h = np.ascontiguousarray(bpe.reshape(4, 128).T)
    wpe_h = np.ascontiguousarray(
        Wpe.reshape(512, 9).reshape(4, 128, 9).transpose(1, 0, 2).reshape(128, 36))
    return dict(wqkvt=wqkvt_h, wprojt=wprojt_h, bqk=bqk_h, bv=bv_h,
                bproj=bproj_h, bpe=bpe_h, wpe=wpe_h,
                ones8=np.ones((128, NH), dtype=ml_dtypes.bfloat16))


def kernel(x, Wqkv, bqkv, Wproj, bproj, Wpe, bpe, _trace=False, _trace_kwargs=None):
    B = x.shape[0]
    if "nc" not in _CACHE:
        _CACHE["nc"] = build()
    nc = _CACHE["nc"]
    shared = _prep(Wqkv, bqkv, Wproj, bproj, Wpe, bpe)
    xb = np.ascontiguousarray(x.reshape(B, C, N)).astype(ml_dtypes.bfloat16)
    in_maps = [dict(shared, x=xb[b]) for b in range(B)]
    res = run_bass_kernel_spmd(nc, in_maps, core_ids=list(range(8)),
                               trace=_trace, **(_trace_kwargs or {}))
    out = np.stack([res.results[b]["out"] for b in range(B)])
    kernel.last_result = res
    return out.reshape(B, C, H, W).astype(np.float32)


# revision 36
# speedup vs baseline: 1.2387x; 1.2387x over previous
"""Trainium2 Bass kernel for nn_Attention_16707422781936.

Data-parallel over batch: B=8 -> one batch element per NeuronCore (8 cores).
Per core: qkv 1x1-conv GEMM, 8-head softmax attention over N=1600 tokens,
proj GEMM, depthwise 3x3 positional-encoding conv, summed output.
"""
import sys

sys.path.insert(0, "/opt/trn_rl_repo")

import ml_dtypes
import numpy as np

import concourse.bass as bass
import concourse.mybir as mybir
import concourse.tile as tile
from concourse import bacc
from concourse.bass_utils import run_bass_kernel_spmd

F32 = mybir.dt.float32
F32R = mybir.dt.float32r
BF16 = mybir.dt.bfloat16
ALU = mybir.AluOpType
EXP = mybir.ActivationFunctionType.Exp

C = 512          # channels
N = 1600         # tokens (40*40)
H = W = 40
NH = 8           # heads
KD = 32          # key dim
HD = 64          # head dim (v)
SCALE = KD ** -0.5

# n blocks (psum-bank sized) and m tiles (partition sized)
NB = [(0, 512), (512, 512), (1024, 512), (1536, 64)]
MT = [(i * 128, min(128, N - i * 128)) for i in range(13)]

_CACHE = {}


def build():
    nc = bacc.Bacc("TRN2", target_bir_lowering=False, debug=False,
                   enable_asserts=False)

    x_d = nc.dram_tensor("x", [C, N], BF16, kind="ExternalInput").ap()
    wqkvt_d = nc.dram_tensor("wqkvt", [128, 4 * 1024], BF16, kind="ExternalInput").ap()
    wprojt_d = nc.dram_tensor("wprojt", [128, 4 * 512], BF16, kind="ExternalInput").ap()
    bqk_d = nc.dram_tensor("bqk", [128, 4], F32, kind="ExternalInput").ap()
    bv_d = nc.dram_tensor("bv", [128, 4], F32, kind="ExternalInput").ap()
    bproj_d = nc.dram_tensor("bproj", [128, 4], F32, kind="ExternalInput").ap()
    bpe_d = nc.dram_tensor("bpe", [128, 4], F32, kind="ExternalInput").ap()
    wpe_d = nc.dram_tensor("wpe", [128, 36], F32, kind="ExternalInput").ap()
    ones_d = nc.dram_tensor("ones8", [128, NH], BF16, kind="ExternalInput").ap()
    out_d = nc.dram_tensor("out", [C, N], F32, kind="ExternalOutput").ap()

    with tile.TileContext(nc) as tc:
        with tc.tile_pool(name="persist", bufs=1) as per:
            qk_sb = per.tile([128, 4, N], BF16, tag="qk")      # q(h0-3),q(h4-7),k(h0-3),k(h4-7)
            v_sb = per.tile([128, 4, N], BF16, tag="v")         # v, channel-major (for dwconv)
            vt_sb = per.tile([128, 13, NH, HD + 1], BF16, tag="vt")  # v^T + ones col
            attn_sb = per.tile([128, 4, N], BF16, tag="attn")  # attention out, channel-major
            wprojt_sb = per.tile([128, 4, 512], BF16, tag="wprojt")
            bqk_sb = per.tile([128, 4], F32, tag="bqk")
            bv_sb = per.tile([128, 4], F32, tag="bv")
            bproj_sb = per.tile([128, 4], F32, tag="bproj")
            bpe_sb = per.tile([128, 4], F32, tag="bpe")
            wpe_sb = per.tile([128, 36], F32, tag="wpe")

            nc.sync.dma_start(wprojt_sb[:], wprojt_d.rearrange("p (t o) -> p t o", t=4))
            nc.sync.dma_start(bqk_sb[:], bqk_d)
            nc.sync.dma_start(bv_sb[:], bv_d)
            nc.sync.dma_start(bproj_sb[:], bproj_d)
            nc.sync.dma_start(bpe_sb[:], bpe_d)
            nc.sync.dma_start(wpe_sb[:], wpe_d)
            for mi in range(13):
                nc.sync.dma_start(vt_sb[:, mi, :, HD:HD + 1],
                                  ones_d.rearrange("p (h o) -> p h o", o=1))

            # ---------------- qkv GEMM phase ----------------
            with tc.tile_pool(name="ph_in", bufs=1) as ph_in:
                x_sb = ph_in.tile([128, 4, N], BF16, tag="x")
                wqkvt_sb = ph_in.tile([128, 4, 1024], BF16, tag="wqkvt")
                x_dr = x_d.rearrange("(t p) n -> p t n", p=128)
                w_dr = wqkvt_d.rearrange("p (t o) -> p t o", t=4)
                for kt in range(4):
                    nc.sync.dma_start(wqkvt_sb[:, kt], w_dr[:, kt])
                    for (n0, nw) in NB:
                        nc.sync.dma_start(x_sb[:, kt, n0:n0 + nw],
                                          x_dr[:, kt, n0:n0 + nw])

                with tc.tile_pool(name="ps_qkv", bufs=4, space="PSUM") as psq:
                    # q and k rows (qkv rows 0:512), output [o, n] layout
                    for mt in range(4):
                        for (n0, nw) in NB:
                            qp = psq.tile([128, 512], F32, tag="qp")
                            for kt in range(4):
                                nc.tensor.matmul(
                                    qp[0:128, 0:nw],
                                    wqkvt_sb[:, kt, mt * 128:(mt + 1) * 128],
                                    x_sb[:, kt, n0:n0 + nw],
                                    start=(kt == 0), stop=(kt == 3))
                            nc.vector.tensor_scalar(
                                out=qk_sb[:, mt, n0:n0 + nw], in0=qp[0:128, 0:nw],
                                scalar1=bqk_sb[:, mt:mt + 1], scalar2=None, op0=ALU.add)
                    # v rows (qkv rows 512:1024), channel-major, +bias (for dwconv)
                    for ct in range(4):
                        for (n0, nw) in NB:
                            qp = psq.tile([128, 512], F32, tag="qp")
                            for kt in range(4):
                                nc.tensor.matmul(
                                    qp[0:128, 0:nw],
                                    wqkvt_sb[:, kt, 512 + ct * 128:512 + (ct + 1) * 128],
                                    x_sb[:, kt, n0:n0 + nw],
                                    start=(kt == 0), stop=(kt == 3))
                            nc.vector.tensor_scalar(
                                out=v_sb[:, ct, n0:n0 + nw], in0=qp[0:128, 0:nw],
                                scalar1=bv_sb[:, ct:ct + 1], scalar2=None, op0=ALU.add)
                    # v^T (no bias; bias folded in post-AV add), token-major
                    for mi, (m0, mw) in enumerate(MT):
                        vp = psq.tile([128, 512], F32, tag="vp")
                        for kt in range(4):
                            nc.tensor.matmul(
                                vp[0:mw, 0:512],
                                x_sb[:, kt, m0:m0 + mw],
                                wqkvt_sb[:, kt, 512:1024],
                                start=(kt == 0), stop=(kt == 3))
                        nc.vector.tensor_copy(
                            out=vt_sb[0:mw, mi, :, 0:HD],
                            in_=vp[0:mw, 0:512].rearrange("p (h d) -> p h d", h=NH))

            # ---------------- depthwise 3x3 conv (VectorE) ----------------
            with tc.tile_pool(name="ph2", bufs=1) as ph2:
                pe_sb = ph2.tile([128, 4, H, W], F32, tag="pe")
                v4 = v_sb[:].rearrange("p t (h w) -> p t h w", h=H)

                # dwconv ops, generated lazily and dripped into attention
                def dwconv_ops():
                    for ct in range(4):
                        def center(ct=ct):
                            nc.vector.tensor_scalar(
                                out=pe_sb[:, ct], in0=v4[:, ct],
                                scalar1=wpe_sb[:, ct * 9 + 4:ct * 9 + 5],
                                scalar2=bpe_sb[:, ct:ct + 1],
                                op0=ALU.mult, op1=ALU.add)
                        yield center
                        for t in range(9):
                            dy, dx = t // 3 - 1, t % 3 - 1
                            if dy == 0 and dx == 0:
                                continue

                            def tap(ct=ct, t=t, dy=dy, dx=dx):
                                ys, ye = max(0, -dy), H - max(0, dy)
                                xs, xe = max(0, -dx), W - max(0, dx)
                                acc = pe_sb[:, ct, ys:ye, xs:xe]
                                nc.vector.scalar_tensor_tensor(
                                    out=acc,
                                    in0=v4[:, ct, ys + dy:ye + dy, xs + dx:xe + dx],
                                    scalar=wpe_sb[:, ct * 9 + t:ct * 9 + t + 1],
                                    in1=acc, op0=ALU.mult, op1=ALU.add)
                            yield tap

                # ---------------- attention + proj (flat pipeline) ----------
                pe3 = pe_sb[:].rearrange("p t h w -> p t (h w)")
                out_dr = out_d.rearrange("(t p) n -> p t n", p=128)
                with tc.tile_pool(name="ps_s", bufs=2, space="PSUM") as pss, \
                     tc.tile_pool(name="ps_av", bufs=4, space="PSUM") as psav, \
                     tc.tile_pool(name="expp", bufs=3) as expp, \
                     tc.tile_pool(name="nrm", bufs=4) as nrm, \
                     tc.tile_pool(name="outp", bufs=3) as outp:

                    def proj_ops(nbi):
                        n0, nw = NB[nbi]
                        for ot in range(4):
                            pp = [None]
                            for kt in range(4):
                                def mm(ot=ot, kt=kt, pp=pp):
                                    if kt == 0:
                                        pp[0] = psav.tile([128, 512], F32, tag="av", name="pp")
                                    nc.tensor.matmul(
                                        pp[0][0:128, 0:nw],
                                        wprojt_sb[:, kt, ot * 128:(ot + 1) * 128],
                                        attn_sb[:, kt, n0:n0 + nw],
                                        start=(kt == 0), stop=(kt == 3))
                                yield mm

                            def evac(ot=ot, pp=pp):
                                ob = outp.tile([128, 512], F32, tag="ob")
                                nc.vector.scalar_tensor_tensor(
                                    out=ob[0:128, 0:nw], in0=pp[0][0:128, 0:nw],
                                    scalar=bproj_sb[:, ot:ot + 1],
                                    in1=pe3[:, ot, n0:n0 + nw],
                                    op0=ALU.add, op1=ALU.add)
                                nc.sync.dma_start(out_dr[:, ot, n0:n0 + nw],
                                                  ob[0:128, 0:nw])
                            yield evac

                    def normalize(p, n0, nw, avs):
                        for j in range(2):
                            drow = nrm.tile([1, 512], F32, tag="drow")
                            dsplit = nrm.tile([32, 16], F32, tag="dsplit")
                            rsplit = nrm.tile([32, 16], F32, tag="rsplit")
                            rc = nrm.tile([1, 512], F32, tag="rc")
                            rb = nrm.tile([HD, 512], F32, tag="rb")
                            nws = nw // 32
                            nc.vector.tensor_copy(drow[0:1, 0:nw],
                                                  avs[j][HD:HD + 1, 0:nw])
                            nc.sync.dma_start(dsplit[0:32, 0:nws], drow[0:1, 0:nw])
                            nc.vector.reciprocal(rsplit[0:32, 0:nws], dsplit[0:32, 0:nws])
                            nc.sync.dma_start(rc[0:1, 0:nw], rsplit[0:32, 0:nws])
                            nc.gpsimd.partition_broadcast(rb[0:HD, 0:nw], rc[0:1, 0:nw])
                            nc.vector.scalar_tensor_tensor(
                                out=attn_sb[j * 64:j * 64 + 64, p, n0:n0 + nw],
                                in0=avs[j][0:HD, 0:nw], scalar=1.0, in1=rb[0:HD, 0:nw],
                                op0=ALU.bypass, op1=ALU.mult)

                    import collections as _c
                    drip = _c.deque(dwconv_ops())   # PE-free DVE drips
                    pe_drip = _c.deque()            # PE drips (proj matmuls)
                    SWEEPS = [(nbi, p) for nbi in range(4) for p in range(4)]
                    pend = None  # deferred AV step: dict of sweep-step state

                    def emit_av(st):
                        for j in range(2):
                            nc.tensor.matmul(
                                st["avs"][j][0:HD + 1, 0:st["nw"]],
                                vt_sb[0:st["mw"], st["mi"], 2 * st["p"] + j, :],
                                st["es"][0:st["mw"], j * 512:j * 512 + st["nw"]],
                                start=(st["mi"] == 0), stop=(st["mi"] == 12))

                    proj_delay = _c.deque()

                    def retire(st):
                        emit_av(st)
                        if st["mi"] == 12:          # sweep finished
                            normalize(st["p"], st["n0"], st["nw"], st["avs"])
                            # release the previous nb's proj drip now that its
                            # normalize chains have had a sweep to complete
                            while proj_delay:
                                pe_drip.append(proj_delay.popleft())
                            if st["p"] == 3:        # all pairs done at this nb
                                proj_delay.extend(proj_ops(st["nbi"]))

                    for (nbi, p) in SWEEPS:
                        n0, nw = NB[nbi]
                        tq = p // 2
                        pb = (p % 2) * 64
                        avs = (psav.tile([HD + 1, 512], F32, tag="av", name="av0"),
                               psav.tile([HD + 1, 512], F32, tag="av", name="av1"))
                        for mi, (m0, mw) in enumerate(MT):
                            sp = pss.tile([128, 1024], F32, tag="sp")
                            for j in range(2):
                                nc.tensor.matmul(
                                    sp[0:mw, j * 512:j * 512 + nw],
                                    qk_sb[pb + 32 * j:pb + 32 * j + 32, 2 + tq, m0:m0 + mw],
                                    qk_sb[pb + 32 * j:pb + 32 * j + 32, tq, n0:n0 + nw],
                                    start=True, stop=True,
                                    tile_position=(pb + 32 * j, 0))
                            es = expp.tile([128, 1024], BF16, tag="es")
                            if nw == 512:
                                nc.scalar.activation(es[0:mw, :], sp[0:mw, :], EXP, scale=SCALE)
                            else:
                                sp3 = sp[:].rearrange("p (j n) -> p j n", j=2)
                                es3 = es[:].rearrange("p (j n) -> p j n", j=2)
                                nc.scalar.activation(es3[0:mw, :, 0:nw], sp3[0:mw, :, 0:nw],
                                                     EXP, scale=SCALE)
                            if pend is not None:
                                retire(pend)
                            if pe_drip:
                                pe_drip.popleft()()
                            elif drip:
                                drip.popleft()()
                            pend = dict(avs=avs, p=p, nw=nw, mi=mi, mw=mw,
                                        es=es, n0=n0, nbi=nbi)
                    retire(pend)
                    while proj_delay:
                        pe_drip.append(proj_delay.popleft())
                    while pe_drip:
                        pe_drip.popleft()()
                    while drip:
                        drip.popleft()()

    nc.compile()
    return nc


def _prep(Wqkv, bqkv, Wproj, bproj, Wpe, bpe):
    WqkvT = np.ascontiguousarray(Wqkv.T)            # [512, 1024]
    wqkvt_h = np.ascontiguousarray(
        WqkvT.reshape(4, 128, 1024).transpose(1, 0, 2).reshape(128, 4096)
    ).astype(ml_dtypes.bfloat16)
    WprojT = np.ascontiguousarray(Wproj.T)          # [512, 512]
    wprojt_h = np.ascontiguousarray(
        WprojT.reshape(4, 128, 512).transpose(1, 0, 2).reshape(128, 2048)
    ).astype(ml_dtypes.bfloat16)
    bqk_h = np.ascontiguousarray(bqkv[0:512].reshape(4, 128).T)
    bv_h = np.ascontiguousarray(bqkv[512:1024].reshape(4, 128).T)
    # attention out is produced WITHOUT the v bias; Wproj @ bv is a constant
    # per output channel, so fold it into the proj bias on the host
    bproj_eff = bproj + Wproj @ bqkv[512:1024]
    bproj_h = np.ascontiguousarray(bproj_eff.reshape(4, 128).T)
    bpe_h = np.ascontiguousarray(bpe.reshape(4, 128).T)
    wpe_h = np.ascontiguousarray(
        Wpe.reshape(512, 9).reshape(4, 128, 9).transpose(1, 0, 2).reshape(128, 36))
    return dict(wqkvt=wqkvt_h, wprojt=wprojt_h, bqk=bqk_h, bv=bv_h,
                bproj=bproj_h, bpe=bpe_h, wpe=wpe_h,
                ones8=np.ones((128, NH), dtype=ml_dtypes.bfloat16))


def kernel(x, Wqkv, bqkv, Wproj, bproj, Wpe, bpe, _trace=False, _trace_kwargs=None):
    B = x.shape[0]
    if "nc" not in _CACHE:
        _CACHE["nc"] = build()
    nc = _CACHE["nc"]
    shared = _prep(Wqkv, bqkv, Wproj, bproj, Wpe, bpe)
    xb = np.ascontiguousarray(x.reshape(B, C, N)).astype(ml_dtypes.bfloat16)
    in_maps = [dict(shared, x=xb[b]) for b in range(B)]
    res = run_bass_kernel_spmd(nc, in_maps, core_ids=list(range(8)),
                               trace=_trace, **(_trace_kwargs or {}))
    out = np.stack([res.results[b]["out"] for b in range(B)])
    kernel.last_result = res
    return out.reshape(B, C, H, W).astype(np.float32)


# revision 37
# speedup vs baseline: 1.2440x; 1.0042x over previous
"""Trainium2 Bass kernel for nn_Attention_16707422781936.

Data-parallel over batch: B=8 -> one batch element per NeuronCore (8 cores).
Per core: qkv 1x1-conv GEMM, 8-head softmax attention over N=1600 tokens,
proj GEMM, depthwise 3x3 positional-encoding conv, summed output.
"""
import sys

sys.path.insert(0, "/opt/trn_rl_repo")

import ml_dtypes
import numpy as np

import concourse.bass as bass
import concourse.mybir as mybir
import concourse.tile as tile
from concourse import bacc
from concourse.bass_utils import run_bass_kernel_spmd

F32 = mybir.dt.float32
F32R = mybir.dt.float32r
BF16 = mybir.dt.bfloat16
ALU = mybir.AluOpType
EXP = mybir.ActivationFunctionType.Exp

C = 512          # channels
N = 1600         # tokens (40*40)
H = W = 40
NH = 8           # heads
KD = 32          # key dim
HD = 64          # head dim (v)
SCALE = KD ** -0.5

# n blocks (psum-bank sized) and m tiles (partition sized)
NB = [(0, 512), (512, 512), (1024, 512), (1536, 64)]
MT = [(i * 128, min(128, N - i * 128)) for i in range(13)]

_CACHE = {}


def build():
    nc = bacc.Bacc("TRN2", target_bir_lowering=False, debug=False,
                   enable_asserts=False)

    x_d = nc.dram_tensor("x", [C, N], BF16, kind="ExternalInput").ap()
    wqkvt_d = nc.dram_tensor("wqkvt", [128, 4 * 1024], BF16, kind="ExternalInput").ap()
    wprojt_d = nc.dram_tensor("wprojt", [128, 4 * 512], BF16, kind="ExternalInput").ap()
    bqk_d = nc.dram_tensor("bqk", [128, 4], F32, kind="ExternalInput").ap()
    bv_d = nc.dram_tensor("bv", [128, 4], F32, kind="ExternalInput").ap()
    bproj_d = nc.dram_tensor("bproj", [128, 4], F32, kind="ExternalInput").ap()
    bpe_d = nc.dram_tensor("bpe", [128, 4], F32, kind="ExternalInput").ap()
    wpe_d = nc.dram_tensor("wpe", [128, 36], F32, kind="ExternalInput").ap()
    ones_d = nc.dram_tensor("ones8", [128, NH], BF16, kind="ExternalInput").ap()
    out_d = nc.dram_tensor("out", [C, N], F32, kind="ExternalOutput").ap()

    with tile.TileContext(nc) as tc:
        with tc.tile_pool(name="persist", bufs=1) as per:
            qk_sb = per.tile([128, 4, N], BF16, tag="qk")      # q(h0-3),q(h4-7),k(h0-3),k(h4-7)
            v_sb = per.tile([128, 4, N], BF16, tag="v")         # v, channel-major (for dwconv)
            vt_sb = per.tile([128, 13, NH, HD + 1], BF16, tag="vt")  # v^T + ones col
            attn_sb = per.tile([128, 4, N], BF16, tag="attn")  # attention out, channel-major
            wprojt_sb = per.tile([128, 4, 512], BF16, tag="wprojt")
            bqk_sb = per.tile([128, 4], F32, tag="bqk")
            bv_sb = per.tile([128, 4], F32, tag="bv")
            bproj_sb = per.tile([128, 4], F32, tag="bproj")
            bpe_sb = per.tile([128, 4], F32, tag="bpe")
            wpe_sb = per.tile([128, 36], F32, tag="wpe")

            nc.sync.dma_start(wprojt_sb[:], wprojt_d.rearrange("p (t o) -> p t o", t=4))
            nc.sync.dma_start(bqk_sb[:], bqk_d)
            nc.sync.dma_start(bv_sb[:], bv_d)
            nc.sync.dma_start(bproj_sb[:], bproj_d)
            nc.sync.dma_start(bpe_sb[:], bpe_d)
            nc.sync.dma_start(wpe_sb[:], wpe_d)
            for mi in range(13):
                nc.sync.dma_start(vt_sb[:, mi, :, HD:HD + 1],
                                  ones_d.rearrange("p (h o) -> p h o", o=1))

            # ---------------- qkv GEMM phase ----------------
            with tc.tile_pool(name="ph_in", bufs=1) as ph_in:
                x_sb = ph_in.tile([128, 4, N], BF16, tag="x")
                wqkvt_sb = ph_in.tile([128, 4, 1024], BF16, tag="wqkvt")
                x_dr = x_d.rearrange("(t p) n -> p t n", p=128)
                w_dr = wqkvt_d.rearrange("p (t o) -> p t o", t=4)
                for kt in range(4):
                    nc.sync.dma_start(wqkvt_sb[:, kt], w_dr[:, kt])
                    for (n0, nw) in NB:
                        nc.sync.dma_start(x_sb[:, kt, n0:n0 + nw],
                                          x_dr[:, kt, n0:n0 + nw])

                with tc.tile_pool(name="ps_qkv", bufs=4, space="PSUM") as psq:
                    # q and k rows (qkv rows 0:512), output [o, n] layout
                    for mt in range(4):
                        for (n0, nw) in NB:
                            qp = psq.tile([128, 512], F32, tag="qp")
                            for kt in range(4):
                                nc.tensor.matmul(
                                    qp[0:128, 0:nw],
                                    wqkvt_sb[:, kt, mt * 128:(mt + 1) * 128],
                                    x_sb[:, kt, n0:n0 + nw],
                                    start=(kt == 0), stop=(kt == 3))
                            nc.vector.tensor_scalar(
                                out=qk_sb[:, mt, n0:n0 + nw], in0=qp[0:128, 0:nw],
                                scalar1=bqk_sb[:, mt:mt + 1], scalar2=None, op0=ALU.add)
                    # v rows (qkv rows 512:1024), channel-major, +bias (for dwconv)
                    for ct in range(4):
                        for (n0, nw) in NB:
                            qp = psq.tile([128, 512], F32, tag="qp")
                            for kt in range(4):
                                nc.tensor.matmul(
                                    qp[0:128, 0:nw],
                                    wqkvt_sb[:, kt, 512 + ct * 128:512 + (ct + 1) * 128],
                                    x_sb[:, kt, n0:n0 + nw],
                                    start=(kt == 0), stop=(kt == 3))
                            nc.vector.tensor_scalar(
                                out=v_sb[:, ct, n0:n0 + nw], in0=qp[0:128, 0:nw],
                                scalar1=bv_sb[:, ct:ct + 1], scalar2=None, op0=ALU.add)
                    # v^T (no bias; bias folded in post-AV add), token-major
                    for mi, (m0, mw) in enumerate(MT):
                        vp = psq.tile([128, 512], F32, tag="vp")
                        for kt in range(4):
                            nc.tensor.matmul(
                                vp[0:mw, 0:512],
                                x_sb[:, kt, m0:m0 + mw],
                                wqkvt_sb[:, kt, 512:1024],
                                start=(kt == 0), stop=(kt == 3))
                        nc.vector.tensor_copy(
                            out=vt_sb[0:mw, mi, :, 0:HD],
                            in_=vp[0:mw, 0:512].rearrange("p (h d) -> p h d", h=NH))

            # ---------------- depthwise 3x3 conv (VectorE) ----------------
            with tc.tile_pool(name="ph2", bufs=1) as ph2:
                pe_sb = ph2.tile([128, 4, H, W], F32, tag="pe")
                v4 = v_sb[:].rearrange("p t (h w) -> p t h w", h=H)

                # dwconv ops, generated lazily and dripped into attention
                def dwconv_ops():
                    for ct in range(4):
                        def center(ct=ct):
                            nc.vector.tensor_scalar(
                                out=pe_sb[:, ct], in0=v4[:, ct],
                                scalar1=wpe_sb[:, ct * 9 + 4:ct * 9 + 5],
                                scalar2=bpe_sb[:, ct:ct + 1],
                                op0=ALU.mult, op1=ALU.add)
                        yield center
                        for t in range(9):
                            dy, dx = t // 3 - 1, t % 3 - 1
                            if dy == 0 and dx == 0:
                                continue

                            def tap(ct=ct, t=t, dy=dy, dx=dx):
                                ys, ye = max(0, -dy), H - max(0, dy)
                                xs, xe = max(0, -dx), W - max(0, dx)
                                acc = pe_sb[:, ct, ys:ye, xs:xe]
                                nc.vector.scalar_tensor_tensor(
                                    out=acc,
                                    in0=v4[:, ct, ys + dy:ye + dy, xs + dx:xe + dx],
                                    scalar=wpe_sb[:, ct * 9 + t:ct * 9 + t + 1],
                                    in1=acc, op0=ALU.mult, op1=ALU.add)
                            yield tap

                # ---------------- attention + proj (flat pipeline) ----------
                pe3 = pe_sb[:].rearrange("p t h w -> p t (h w)")
                out_dr = out_d.rearrange("(t p) n -> p t n", p=128)
                with tc.tile_pool(name="ps_s", bufs=2, space="PSUM") as pss, \
                     tc.tile_pool(name="ps_av", bufs=4, space="PSUM") as psav, \
                     tc.tile_pool(name="expp", bufs=4) as expp, \
                     tc.tile_pool(name="nrm", bufs=4) as nrm, \
                     tc.tile_pool(name="outp", bufs=3) as outp:

                    def proj_ops(nbi):
                        n0, nw = NB[nbi]
                        for ot in range(4):
                            pp = [None]
                            for kt in range(4):
                                def mm(ot=ot, kt=kt, pp=pp):
                                    if kt == 0:
                                        pp[0] = psav.tile([128, 512], F32, tag="av", name="pp")
                                    nc.tensor.matmul(
                                        pp[0][0:128, 0:nw],
                                        wprojt_sb[:, kt, ot * 128:(ot + 1) * 128],
                                        attn_sb[:, kt, n0:n0 + nw],
                                        start=(kt == 0), stop=(kt == 3))
                                yield mm

                            def evac(ot=ot, pp=pp):
                                ob = outp.tile([128, 512], F32, tag="ob")
                                nc.vector.scalar_tensor_tensor(
                                    out=ob[0:128, 0:nw], in0=pp[0][0:128, 0:nw],
                                    scalar=bproj_sb[:, ot:ot + 1],
                                    in1=pe3[:, ot, n0:n0 + nw],
                                    op0=ALU.add, op1=ALU.add)
                                nc.sync.dma_start(out_dr[:, ot, n0:n0 + nw],
                                                  ob[0:128, 0:nw])
                            yield evac

                    def normalize(p, n0, nw, avs):
                        for j in range(2):
                            drow = nrm.tile([1, 512], F32, tag="drow")
                            dsplit = nrm.tile([32, 16], F32, tag="dsplit")
                            rsplit = nrm.tile([32, 16], F32, tag="rsplit")
                            rc = nrm.tile([1, 512], F32, tag="rc")
                            rb = nrm.tile([HD, 512], F32, tag="rb")
                            nws = nw // 32
                            nc.vector.tensor_copy(drow[0:1, 0:nw],
                                                  avs[j][HD:HD + 1, 0:nw])
                            nc.sync.dma_start(dsplit[0:32, 0:nws], drow[0:1, 0:nw])
                            nc.vector.reciprocal(rsplit[0:32, 0:nws], dsplit[0:32, 0:nws])
                            nc.sync.dma_start(rc[0:1, 0:nw], rsplit[0:32, 0:nws])
                            nc.gpsimd.partition_broadcast(rb[0:HD, 0:nw], rc[0:1, 0:nw])
                            nc.vector.scalar_tensor_tensor(
                                out=attn_sb[j * 64:j * 64 + 64, p, n0:n0 + nw],
                                in0=avs[j][0:HD, 0:nw], scalar=1.0, in1=rb[0:HD, 0:nw],
                                op0=ALU.bypass, op1=ALU.mult)

                    import collections as _c
                    drip = _c.deque(dwconv_ops())   # PE-free DVE drips
                    pe_drip = _c.deque()            # PE drips (proj matmuls)
                    SWEEPS = [(nbi, p) for nbi in range(4) for p in range(4)]
                    pend = None  # deferred AV step: dict of sweep-step state

                    def emit_av(st):
                        for j in range(2):
                            nc.tensor.matmul(
                                st["avs"][j][0:HD + 1, 0:st["nw"]],
                                vt_sb[0:st["mw"], st["mi"], 2 * st["p"] + j, :],
                                st["es"][0:st["mw"], j * 512:j * 512 + st["nw"]],
                                start=(st["mi"] == 0), stop=(st["mi"] == 12))

                    proj_delay = _c.deque()

                    def retire(st):
                        emit_av(st)
                        if st["mi"] == 12:          # sweep finished
                            normalize(st["p"], st["n0"], st["nw"], st["avs"])
                            # release the previous nb's proj drip now that its
                            # normalize chains have had a sweep to complete
                            while proj_delay:
                                pe_drip.append(proj_delay.popleft())
                            if st["p"] == 3:        # all pairs done at this nb
                                proj_delay.extend(proj_ops(st["nbi"]))

                    for (nbi, p) in SWEEPS:
                        n0, nw = NB[nbi]
                        tq = p // 2
                        pb = (p % 2) * 64
                        avs = (psav.tile([HD + 1, 512], F32, tag="av", name="av0"),
                               psav.tile([HD + 1, 512], F32, tag="av", name="av1"))
                        for mi, (m0, mw) in enumerate(MT):
                            sp = pss.tile([128, 1024], F32, tag="sp")
                            for j in range(2):
                                nc.tensor.matmul(
                                    sp[0:mw, j * 512:j * 512 + nw],
                                    qk_sb[pb + 32 * j:pb + 32 * j + 32, 2 + tq, m0:m0 + mw],
                                    qk_sb[pb + 32 * j:pb + 32 * j + 32, tq, n0:n0 + nw],
                                    start=True, stop=True,
                                    tile_position=(pb + 32 * j, 0))
                            es = expp.tile([128, 1024], BF16, tag="es")
                            if nw == 512:
                                nc.scalar.activation(es[0:mw, :], sp[0:mw, :], EXP, scale=SCALE)
                            else:
                                sp3 = sp[:].rearrange("p (j n) -> p j n", j=2)
                                es3 = es[:].rearrange("p (j n) -> p j n", j=2)
                                nc.scalar.activation(es3[0:mw, :, 0:nw], sp3[0:mw, :, 0:nw],
                                                     EXP, scale=SCALE)
                            if pend is not None:
                                retire(pend)
                            if pe_drip:
                                pe_drip.popleft()()
                            elif drip:
                                drip.popleft()()
                            pend = dict(avs=avs, p=p, nw=nw, mi=mi, mw=mw,
                                        es=es, n0=n0, nbi=nbi)
                    retire(pend)
                    while proj_delay:
                        pe_drip.append(proj_delay.popleft())
                    while pe_drip:
                        pe_drip.popleft()()
                    while drip:
                        drip.popleft()()

    nc.compile()
    return nc


def _prep(Wqkv, bqkv, Wproj, bproj, Wpe, bpe):
    WqkvT = np.ascontiguousarray(Wqkv.T)            # [512, 1024]
    wqkvt_h = np.ascontiguousarray(
        WqkvT.reshape(4, 128, 1024).transpose(1, 0, 2).reshape(128, 4096)
    ).astype(ml_dtypes.bfloat16)
    WprojT = np.ascontiguousarray(Wproj.T)          # [512, 512]
    wprojt_h = np.ascontiguousarray(
        WprojT.reshape(4, 128, 512).transpose(1, 0, 2).reshape(128, 2048)
    ).astype(ml_dtypes.bfloat16)
    bqk_h = np.ascontiguousarray(bqkv[0:512].reshape(4, 128).T)
    bv_h = np.ascontiguousarray(bqkv[512:1024].reshape(4, 128).T)
    # attention out is produced WITHOUT the v bias; Wproj @ bv is a constant
    # per output channel, so fold it into the proj bias on the host
    bproj_eff = bproj + Wproj @ bqkv[512:1024]
    bproj_h = np.ascontiguousarray(bproj_eff.reshape(4, 128).T)
    bpe_h = np.ascontiguousarray(bpe.reshape(4, 128).T)
    wpe_h = np.ascontiguousarray(
        Wpe.reshape(512, 9).reshape(4, 128, 9).transpose(1, 0, 2).reshape(128, 36))
    return dict(wqkvt=wqkvt_h, wprojt=wprojt_h, bqk=bqk_h, bv=bv_h,
                bproj=bproj_h, bpe=bpe_h, wpe=wpe_h,
                ones8=np.ones((128, NH), dtype=ml_dtypes.bfloat16))


def kernel(x, Wqkv, bqkv, Wproj, bproj, Wpe, bpe, _trace=False, _trace_kwargs=None):
    B = x.shape[0]
    if "nc" not in _CACHE:
        _CACHE["nc"] = build()
    nc = _CACHE["nc"]
    shared = _prep(Wqkv, bqkv, Wproj, bproj, Wpe, bpe)
    xb = np.ascontiguousarray(x.reshape(B, C, N)).astype(ml_dtypes.bfloat16)
    in_maps = [dict(shared, x=xb[b]) for b in range(B)]
    res = run_bass_kernel_spmd(nc, in_maps, core_ids=list(range(8)),
                               trace=_trace, **(_trace_kwargs or {}))
    out = np.stack([res.results[b]["out"] for b in range(B)])
    kernel.last_result = res
    return out.reshape(B, C, H, W).astype(np.float32)


# revision 40
# speedup vs baseline: 1.3150x; 1.0571x over previous
"""Trainium2 Bass kernel for nn_Attention_16707422781936.

Data-parallel over batch: B=8 -> one batch element per NeuronCore (8 cores).
Per core: qkv 1x1-conv GEMM, 8-head softmax attention over N=1600 tokens,
proj GEMM, depthwise 3x3 positional-encoding conv, summed output.
"""
import sys

sys.path.insert(0, "/opt/trn_rl_repo")

import ml_dtypes
import numpy as np

import concourse.bass as bass
import concourse.mybir as mybir
import concourse.tile as tile
from concourse import bacc
from concourse.bass_utils import run_bass_kernel_spmd

F32 = mybir.dt.float32
F32R = mybir.dt.float32r
BF16 = mybir.dt.bfloat16
ALU = mybir.AluOpType
EXP = mybir.ActivationFunctionType.Exp

C = 512          # channels
N = 1600         # tokens (40*40)
H = W = 40
NH = 8           # heads
KD = 32          # key dim
HD = 64          # head dim (v)
SCALE = KD ** -0.5

# n blocks (psum-bank sized) and m tiles (partition sized)
NB = [(0, 512), (512, 512), (1024, 512), (1536, 64)]
MT = [(i * 128, min(128, N - i * 128)) for i in range(13)]

_CACHE = {}


def build():
    nc = bacc.Bacc("TRN2", target_bir_lowering=False, debug=False,
                   enable_asserts=False)

    x_d = nc.dram_tensor("x", [C, N], BF16, kind="ExternalInput").ap()
    wqkvt_d = nc.dram_tensor("wqkvt", [128, 4 * 1024], BF16, kind="ExternalInput").ap()
    wprojt_d = nc.dram_tensor("wprojt", [128, 4 * 512], BF16, kind="ExternalInput").ap()
    bqk_d = nc.dram_tensor("bqk", [128, 4], F32, kind="ExternalInput").ap()
    bv_d = nc.dram_tensor("bv", [128, 4], F32, kind="ExternalInput").ap()
    bproj_d = nc.dram_tensor("bproj", [128, 4], F32, kind="ExternalInput").ap()
    bpe_d = nc.dram_tensor("bpe", [128, 4], F32, kind="ExternalInput").ap()
    wpe_d = nc.dram_tensor("wpe", [128, 36], F32, kind="ExternalInput").ap()
    ones_d = nc.dram_tensor("ones8", [128, NH], BF16, kind="ExternalInput").ap()
    out_d = nc.dram_tensor("out", [C, N], F32, kind="ExternalOutput").ap()

    with tile.TileContext(nc) as tc:
        with tc.tile_pool(name="persist", bufs=1) as per:
            qk_sb = per.tile([128, 4, N], BF16, tag="qk")      # q(h0-3),q(h4-7),k(h0-3),k(h4-7)
            v_sb = per.tile([128, 4, N], BF16, tag="v")         # v, channel-major (for dwconv)
            vt_sb = per.tile([128, 13, NH, HD + 1], BF16, tag="vt")  # v^T + ones col
            attn_sb = per.tile([128, 4, N], BF16, tag="attn")  # attention out, channel-major
            wprojt_sb = per.tile([128, 4, 512], BF16, tag="wprojt")
            bqk_sb = per.tile([128, 4], F32, tag="bqk")
            bv_sb = per.tile([128, 4], F32, tag="bv")
            bproj_sb = per.tile([128, 4], F32, tag="bproj")
            bpe_sb = per.tile([128, 4], F32, tag="bpe")
            wpe_sb = per.tile([128, 36], F32, tag="wpe")

            nc.sync.dma_start(wprojt_sb[:], wprojt_d.rearrange("p (t o) -> p t o", t=4))
            nc.sync.dma_start(bqk_sb[:], bqk_d)
            nc.sync.dma_start(bv_sb[:], bv_d)
            nc.sync.dma_start(bproj_sb[:], bproj_d)
            nc.sync.dma_start(bpe_sb[:], bpe_d)
            nc.sync.dma_start(wpe_sb[:], wpe_d)
            for mi in range(13):
                nc.sync.dma_start(vt_sb[:, mi, :, HD:HD + 1],
                                  ones_d.rearrange("p (h o) -> p h o", o=1))

            # ---------------- qkv GEMM phase (prefix only) ----------------
            # Only what attention pair 0 needs up front: q/k for heads 0-3
            # (mt 0 and 2) and the full v^T. The rest (q/k heads 4-7, v) is
            # dripped into the attention pipeline's PE slack below.
            x_sb = per.tile([128, 4, N], BF16, tag="x")
            wqkvt_sb = per.tile([128, 4, 1024], BF16, tag="wqkvt")
            x_dr = x_d.rearrange("(t p) n -> p t n", p=128)
            w_dr = wqkvt_d.rearrange("p (t o) -> p t o", t=4)
            for kt in range(4):
                nc.sync.dma_start(wqkvt_sb[:, kt], w_dr[:, kt])
            for (n0, nw) in NB:
                for kt in range(4):
                    nc.sync.dma_start(x_sb[:, kt, n0:n0 + nw],
                                      x_dr[:, kt, n0:n0 + nw])

            def qk_group(pool, tag, mt, n0, nw):
                qp = pool.tile([128, 512], F32, tag=tag, name="qg")
                for kt in range(4):
                    nc.tensor.matmul(
                        qp[0:128, 0:nw],
                        wqkvt_sb[:, kt, mt * 128:(mt + 1) * 128],
                        x_sb[:, kt, n0:n0 + nw],
                        start=(kt == 0), stop=(kt == 3))
                nc.vector.tensor_scalar(
                    out=qk_sb[:, mt, n0:n0 + nw], in0=qp[0:128, 0:nw],
                    scalar1=bqk_sb[:, mt:mt + 1], scalar2=None, op0=ALU.add)

            def v_group(pool, tag, ct, n0, nw):
                qp = pool.tile([128, 512], F32, tag=tag, name="vg")
                for kt in range(4):
                    nc.tensor.matmul(
                        qp[0:128, 0:nw],
                        wqkvt_sb[:, kt, 512 + ct * 128:512 + (ct + 1) * 128],
                        x_sb[:, kt, n0:n0 + nw],
                        start=(kt == 0), stop=(kt == 3))
                nc.vector.tensor_scalar(
                    out=v_sb[:, ct, n0:n0 + nw], in0=qp[0:128, 0:nw],
                    scalar1=bv_sb[:, ct:ct + 1], scalar2=None, op0=ALU.add)

            with tc.tile_pool(name="ps_qkv", bufs=4, space="PSUM") as psq:
                for mt in (0, 2):
                    for (n0, nw) in NB:
                        qk_group(psq, "qp", mt, n0, nw)
                for mi, (m0, mw) in enumerate(MT):
                    vp = psq.tile([128, 512], F32, tag="vp")
                    for kt in range(4):
                        nc.tensor.matmul(
                            vp[0:mw, 0:512],
                            x_sb[:, kt, m0:m0 + mw],
                            wqkvt_sb[:, kt, 512:1024],
                            start=(kt == 0), stop=(kt == 3))
                    nc.vector.tensor_copy(
                        out=vt_sb[0:mw, mi, :, 0:HD],
                        in_=vp[0:mw, 0:512].rearrange("p (h d) -> p h d", h=NH))

            # ---------------- depthwise 3x3 conv (VectorE) ----------------
            with tc.tile_pool(name="ph2", bufs=1) as ph2:
                pe_sb = ph2.tile([128, 4, H, W], F32, tag="pe")
                v4 = v_sb[:].rearrange("p t (h w) -> p t h w", h=H)

                # dwconv ops, generated lazily and dripped into attention
                def dwconv_ops():
                    for ct in range(4):
                        def center(ct=ct):
                            nc.vector.tensor_scalar(
                                out=pe_sb[:, ct], in0=v4[:, ct],
                                scalar1=wpe_sb[:, ct * 9 + 4:ct * 9 + 5],
                                scalar2=bpe_sb[:, ct:ct + 1],
                                op0=ALU.mult, op1=ALU.add)
                        yield center
                        for t in range(9):
                            dy, dx = t // 3 - 1, t % 3 - 1
                            if dy == 0 and dx == 0:
                                continue

                            def tap(ct=ct, t=t, dy=dy, dx=dx):
                                ys, ye = max(0, -dy), H - max(0, dy)
                                xs, xe = max(0, -dx), W - max(0, dx)
                                acc = pe_sb[:, ct, ys:ye, xs:xe]
                                nc.vector.scalar_tensor_tensor(
                                    out=acc,
                                    in0=v4[:, ct, ys + dy:ye + dy, xs + dx:xe + dx],
                                    scalar=wpe_sb[:, ct * 9 + t:ct * 9 + t + 1],
                                    in1=acc, op0=ALU.mult, op1=ALU.add)
                            yield tap

                # ---------------- attention + proj (flat pipeline) ----------
                pe3 = pe_sb[:].rearrange("p t h w -> p t (h w)")
                out_dr = out_d.rearrange("(t p) n -> p t n", p=128)
                with tc.tile_pool(name="ps_s", bufs=2, space="PSUM") as pss, \
                     tc.tile_pool(name="ps_av", bufs=4, space="PSUM") as psav, \
                     tc.tile_pool(name="expp", bufs=4) as expp, \
                     tc.tile_pool(name="nrm", bufs=4) as nrm, \
                     tc.tile_pool(name="outp", bufs=3) as outp:

                    def proj_ops(nbi):
                        n0, nw = NB[nbi]
                        for ot in range(4):
                            pp = [None]
                            for kt in range(4):
                                def mm(ot=ot, kt=kt, pp=pp):
                                    if kt == 0:
                                        pp[0] = psav.tile([128, 512], F32, tag="av", name="pp")
                                    nc.tensor.matmul(
                                        pp[0][0:128, 0:nw],
                                        wprojt_sb[:, kt, ot * 128:(ot + 1) * 128],
                                        attn_sb[:, kt, n0:n0 + nw],
                                        start=(kt == 0), stop=(kt == 3))
                                yield mm

                            def evac(ot=ot, pp=pp):
                                ob = outp.tile([128, 512], F32, tag="ob")
                                nc.vector.scalar_tensor_tensor(
                                    out=ob[0:128, 0:nw], in0=pp[0][0:128, 0:nw],
                                    scalar=bproj_sb[:, ot:ot + 1],
                                    in1=pe3[:, ot, n0:n0 + nw],
                                    op0=ALU.add, op1=ALU.add)
                                nc.sync.dma_start(out_dr[:, ot, n0:n0 + nw],
                                                  ob[0:128, 0:nw])
                            yield evac

                    def normalize(p, n0, nw, avs):
                        for j in range(2):
                            drow = nrm.tile([1, 512], F32, tag="drow")
                            dsplit = nrm.tile([32, 16], F32, tag="dsplit")
                            rsplit = nrm.tile([32, 16], F32, tag="rsplit")
                            rc = nrm.tile([1, 512], F32, tag="rc")
                            rb = nrm.tile([HD, 512], F32, tag="rb")
                            nws = nw // 32
                            nc.vector.tensor_copy(drow[0:1, 0:nw],
                                                  avs[j][HD:HD + 1, 0:nw])
                            nc.sync.dma_start(dsplit[0:32, 0:nws], drow[0:1, 0:nw])
                            nc.vector.reciprocal(rsplit[0:32, 0:nws], dsplit[0:32, 0:nws])
                            nc.sync.dma_start(rc[0:1, 0:nw], rsplit[0:32, 0:nws])
                            nc.gpsimd.partition_broadcast(rb[0:HD, 0:nw], rc[0:1, 0:nw])
                            nc.vector.scalar_tensor_tensor(
                                out=attn_sb[j * 64:j * 64 + 64, p, n0:n0 + nw],
                                in0=avs[j][0:HD, 0:nw], scalar=1.0, in1=rb[0:HD, 0:nw],
                                op0=ALU.bypass, op1=ALU.mult)

                    import collections as _c
                    drip = _c.deque(dwconv_ops())   # PE-free DVE drips
                    pe_drip = _c.deque()            # PE drips (proj matmuls)
                    # remaining qkv work, dripped one group per step at top
                    # priority: q/k of heads 4-7 first (needed by pair 2 at
                    # step 26), then v (only dwconv consumes it, later)
                    qkv_drip = _c.deque()
                    for (n0d, nwd) in NB:
                        qkv_drip.append(lambda n0=n0d, nw=nwd: qk_group(psav, "av", 1, n0, nw))
                    for (n0d, nwd) in NB:
                        qkv_drip.append(lambda n0=n0d, nw=nwd: qk_group(psav, "av", 3, n0, nw))
                    for ctd in range(4):
                        for (n0d, nwd) in NB:
                            qkv_drip.append(lambda ct=ctd, n0=n0d, nw=nwd: v_group(psav, "av", ct, n0, nw))
                    SWEEPS = [(nbi, p) for nbi in range(4) for p in range(4)]
                    pend = None  # deferred AV step: dict of sweep-step state

                    def emit_av(st):
                        for j in range(2):
                            nc.tensor.matmul(
                                st["avs"][j][0:HD + 1, 0:st["nw"]],
                                vt_sb[0:st["mw"], st["mi"], 2 * st["p"] + j, :],
                                st["es"][0:st["mw"], j * 512:j * 512 + st["nw"]],
                                start=(st["mi"] == 0), stop=(st["mi"] == 12))

                    proj_delay = _c.deque()

                    def retire(st):
                        emit_av(st)
                        if st["mi"] == 12:          # sweep finished
                            normalize(st["p"], st["n0"], st["nw"], st["avs"])
                            # release the previous nb's proj drip now that its
                            # normalize chains have had a sweep to complete
                            while proj_delay:
                                pe_drip.append(proj_delay.popleft())
                            if st["p"] == 3:        # all pairs done at this nb
                                proj_delay.extend(proj_ops(st["nbi"]))

                    for (nbi, p) in SWEEPS:
                        n0, nw = NB[nbi]
                        tq = p // 2
                        pb = (p % 2) * 64
                        avs = (psav.tile([HD + 1, 512], F32, tag="av", name="av0"),
                               psav.tile([HD + 1, 512], F32, tag="av", name="av1"))
                        for mi, (m0, mw) in enumerate(MT):
                            sp = pss.tile([128, 1024], F32, tag="sp")
                            for j in range(2):
                                nc.tensor.matmul(
                                    sp[0:mw, j * 512:j * 512 + nw],
                                    qk_sb[pb + 32 * j:pb + 32 * j + 32, 2 + tq, m0:m0 + mw],
                                    qk_sb[pb + 32 * j:pb + 32 * j + 32, tq, n0:n0 + nw],
                                    start=True, stop=True,
                                    tile_position=(pb + 32 * j, 0))
                            es = expp.tile([128, 1024], BF16, tag="es")
                            if nw == 512:
                                nc.scalar.activation(es[0:mw, :], sp[0:mw, :], EXP, scale=SCALE)
                            else:
                                sp3 = sp[:].rearrange("p (j n) -> p j n", j=2)
                                es3 = es[:].rearrange("p (j n) -> p j n", j=2)
                                nc.scalar.activation(es3[0:mw, :, 0:nw], sp3[0:mw, :, 0:nw],
                                                     EXP, scale=SCALE)
                            if pend is not None:
                                retire(pend)
                            if qkv_drip:
                                qkv_drip.popleft()()
                            elif pe_drip:
                                pe_drip.popleft()()
                            elif drip:
                                drip.popleft()()
                            pend = dict(avs=avs, p=p, nw=nw, mi=mi, mw=mw,
                                        es=es, n0=n0, nbi=nbi)
                    retire(pend)
                    while proj_delay:
                        pe_drip.append(proj_delay.popleft())
                    while pe_drip:
                        pe_drip.popleft()()
                    while drip:
                        drip.popleft()()

    nc.compile()
    return nc


def _prep(Wqkv, bqkv, Wproj, bproj, Wpe, bpe):
    WqkvT = np.ascontiguousarray(Wqkv.T)            # [512, 1024]
    wqkvt_h = np.ascontiguousarray(
        WqkvT.reshape(4, 128, 1024).transpose(1, 0, 2).reshape(128, 4096)
    ).astype(ml_dtypes.bfloat16)
    WprojT = np.ascontiguousarray(Wproj.T)          # [512, 512]
    wprojt_h = np.ascontiguousarray(
        WprojT.reshape(4, 128, 512).transpose(1, 0, 2).reshape(128, 2048)
    ).astype(ml_dtypes.bfloat16)
    bqk_h = np.ascontiguousarray(bqkv[0:512].reshape(4, 128).T)
    bv_h = np.ascontiguousarray(bqkv[512:1024].reshape(4, 128).T)
    # attention out is produced WITHOUT the v bias; Wproj @ bv is a constant
    # per output channel, so fold it into the proj bias on the host
    bproj_eff = bproj + Wproj @ bqkv[512:1024]
    bproj_h = np.ascontiguousarray(bproj_eff.reshape(4, 128).T)
    bpe_h = np.ascontiguousarray(bpe.reshape(4, 128).T)
    wpe_h = np.ascontiguousarray(
        Wpe.reshape(512, 9).reshape(4, 128, 9).transpose(1, 0, 2).reshape(128, 36))
    return dict(wqkvt=wqkvt_h, wprojt=wprojt_h, bqk=bqk_h, bv=bv_h,
                bproj=bproj_h, bpe=bpe_h, wpe=wpe_h,
                ones8=np.ones((128, NH), dtype=ml_dtypes.bfloat16))


def kernel(x, Wqkv, bqkv, Wproj, bproj, Wpe, bpe, _trace=False, _trace_kwargs=None):
    x = np.asarray(x, dtype=np.float32)
    Wqkv = np.asarray(Wqkv, dtype=np.float32)
    bqkv = np.asarray(bqkv, dtype=np.float32)
    Wproj = np.asarray(Wproj, dtype=np.float32)
    bproj = np.asarray(bproj, dtype=np.float32)
    Wpe = np.asarray(Wpe, dtype=np.float32)
    bpe = np.asarray(bpe, dtype=np.float32)
    B = x.shape[0]
    if "nc" not in _CACHE:
        _CACHE["nc"] = build()
    nc = _CACHE["nc"]
    shared = _prep(Wqkv, bqkv, Wproj, bproj, Wpe, bpe)
    xb = np.ascontiguousarray(x.reshape(B, C, N)).astype(ml_dtypes.bfloat16)
    in_maps = [dict(shared, x=xb[b]) for b in range(B)]
    res = run_bass_kernel_spmd(nc, in_maps, core_ids=list(range(8)),
                               trace=_trace, **(_trace_kwargs or {}))
    out = np.stack([res.results[b]["out"] for b in range(B)])
    kernel.last_result = res
    return out.reshape(B, C, H, W).astype(np.float32)
